# revision 14
# baseline (speedup 1.0000x reference)
"""TRN2 Bass kernel for nn_Attention_70257075028315.

reference:
    scores = einsum('bqd,bkd->bqk', query, key)       # B=8, Nq=Nk=2048, D=512
    probs  = softmax(scores, -1)
    out    = einsum('bqk,bkd->bqd', probs, key)

Sharding: batch b -> NeuronCore b (data parallel, fully local attention).

Per-core program (q/k: [2048, 512] fp32):
  Phase A/B (per 1 MB group of 4 tiles; order K0,Q0,K1..K3,Q1..Q3):
    DMA load fp32 -> cast to fp16 (K on DVE into k_pv, Q on ACT into scratch)
    -> ONE group-wide DMA xbar transpose ([128, 2048] fp16, 128 tiles x 14 ns)
    into kT/qT [128(dp), tile, dc, 128].  The PE does NO transposes at all.
  Phase C (per q-tile, software-pipelined across tiles):
    S     = qT.T @ kT   fp16 matmuls, 4 d-chunk-accumulated per 512-wide
            chunk, each chunk in its OWN PSUM bank tile
    max   per chunk on DVE as soon as the chunk lands; combined, negated
    p     = exp(S - max): one ACT pass per chunk, PSUM -> SBUF fp16, with
            fused per-chunk row-sum accumulation; 1/sum via DVE reciprocal
    pT    = per-chunk DMA xbar transpose (SP-dispatched) right behind each
            exp chunk -> pT [128(kk), 16, 128] fp16, overlapping S(i+1)
    o     = pT.T @ k_pv  16 kk-accumulated fp16 matmuls -> PSUM [128, 512]
    out   = o * (1/rowsum) on DVE, then DMA out.
  Emission order per step i: S(i+1)+E(i+1)+Tdma(i+1), PV(i), with an explicit
  PE-queue dep keeping PV(i) after S(i+1) so PV hides the max->exp->transpose
  latency of tile i+1.  PSUM: 4 banks S chunks + 2 PV accum = 6.

Dtype choices (HW-measured):
- scores fp16 (16-bit operands stream 2 cols/cycle vs f32r's 1): score abs
  err ~3e-2 -> output rel err ~2e-3, well under the 2e-2 gate. bf16 scores
  are fatal (0.27 abs err flips argmaxes in this near-one-hot softmax).
- PV (probs @ K): float16 (10-bit mantissa; bf16 is no faster and 8x less
  accurate). Mixing 16/32-bit matmul operands is rejected (NCC_IBIR034).
- DMA xbar transpose requires 2-byte dtypes; layout [p, m, q] = row m*128+p
  of the transposed matrix (verified on HW), which matches the natural
  k = t*128+kk / d = dc*128+dp layouts used by the matmuls.
"""

import numpy as np

import concourse.bass as bass
import concourse.tile as tile
import concourse.mybir as mybir
from concourse import bacc
from concourse.bass_utils import run_bass_kernel_spmd

FP32 = mybir.dt.float32
FP32R = mybir.dt.float32r
FP16 = mybir.dt.float16
AF = mybir.ActivationFunctionType

B, NQ, NK, D = 8, 2048, 2048, 512
P = 128
NKT = NK // P   # 16 kk tiles
NQT = NQ // P   # 16 q tiles
NDC = D // P    # 4 d chunks
NCH = NK // 512  # 4 score chunks of 512


LOOP_UNROLL = 2  # phase-C emissions per For_i iteration in the timed build


def build(score_dtype=FP16, repeat_c=1, timed=False, pv_dtype=FP16,
          kpv_bf16=False, depth2=False, pt_split=4, ps_s_bufs=6,
          staggered=False, work_bufs=2, loop_unroll=LOOP_UNROLL):
    """timed=True adds an int32 [1,1] input "reps": phase C re-runs in a
    dynamic For_i loop `reps` more times (0 = just the normal kernel), so one
    NEFF can measure the phase-C slope against itself."""
    nc = bacc.Bacc("TRN2", target_bir_lowering=False, debug=False)
    q_d = nc.dram_tensor("query", [NQ, D], FP32, kind="ExternalInput").ap()
    k_d = nc.dram_tensor("key", [NK, D], FP32, kind="ExternalInput").ap()
    reps_d = None
    if timed:
        reps_d = nc.dram_tensor(
            "reps", [1, 1], mybir.dt.int32, kind="ExternalInput"
        ).ap()
    out_d = nc.dram_tensor("out", [NQ, D], FP32, kind="ExternalOutput").ap()

    q_tiles_d = q_d.rearrange("(t p) d -> t p d", p=P)
    k_tiles_d = k_d.rearrange("(t p) d -> t p d", p=P)
    out_tiles_d = out_d.rearrange("(t p) d -> t p d", p=P)

    with tile.TileContext(nc) as tc:
        _body(tc, q_tiles_d, k_tiles_d, out_tiles_d, score_dtype, repeat_c,
              reps_d, pv_dtype, kpv_bf16, depth2, pt_split, ps_s_bufs,
              staggered, work_bufs, loop_unroll)
    nc.compile()
    return nc


def _body(tc, q_tiles_d, k_tiles_d, out_tiles_d, score_dtype, repeat_c,
          reps_d=None, pv_dtype=FP16, kpv_bf16=False, depth2=False,
          pt_split=4, ps_s_bufs=6, staggered=False, work_bufs=2,
          loop_unroll=LOOP_UNROLL):
    from contextlib import ExitStack

    nc = tc.nc
    reps_rv = None
    if reps_d is not None:
        regs = nc.alloc_registers("reps_regs")
        nc.regs_load(regs, reps_d[0:1, 0:1])
        reps_rv = nc.snap(regs, donate=True, min_val=0, max_val=64)
    with ExitStack() as ctx:
        persist = ctx.enter_context(tc.tile_pool(name="persist", bufs=1))
        work = ctx.enter_context(tc.tile_pool(name="work", bufs=work_bufs))
        small = ctx.enter_context(tc.tile_pool(name="small", bufs=3))
        load = ctx.enter_context(tc.tile_pool(name="load", bufs=4))
        ps_s = ctx.enter_context(
            tc.tile_pool(name="ps_s", bufs=ps_s_bufs, space="PSUM"))
        ps_pv = ctx.enter_context(tc.tile_pool(name="ps_pv", bufs=2, space="PSUM"))

        # Transposed operands, [dp, tile, dc, 128] so a group-wide DMA xbar
        # transpose fills 4 tiles in one instruction (middle dims merge).
        kT = persist.tile([P, NKT, NDC, P], score_dtype)
        qT = persist.tile([P, NQT, NDC, P], score_dtype)
        kpv_dt = mybir.dt.bfloat16 if kpv_bf16 else pv_dtype
        k_pv = persist.tile([P, NKT, D // P, P], kpv_dt)  # natural [kk, d]

        # ---- Phase A/B: load fp32, cast to fp16, DMA-xbar-transpose ----
        def emit_load_group(src_d, dstT, pv, g):
            gt = load.tile([P, 4, D], FP32, tag="ld")
            nc.sync.dma_start(
                gt[:], src_d[g * 4 : (g + 1) * 4].rearrange("t p d -> p t d")
            )
            if pv is not None:  # K: cast on DVE into the persistent k_pv
                g16 = pv[:, g * 4 : (g + 1) * 4]
                nc.vector.tensor_copy(
                    g16, gt[:].rearrange("p t (a b) -> p t a b", b=P)
                )
            else:  # Q: cast on ACT into scratch
                g16 = load.tile([P, 4, D // P, P], score_dtype, tag="q16")
                nc.scalar.copy(g16[:], gt[:].rearrange("p t (a b) -> p t a b", b=P))
            nc.scalar.dma_start_transpose(
                dstT[:, g * 4 : (g + 1) * 4],
                g16.rearrange("p t a b -> p (t a b)"),
            )

        emit_load_group(k_tiles_d, kT, k_pv, 0)
        emit_load_group(q_tiles_d, qT, None, 0)
        for g in range(1, 4):
            emit_load_group(k_tiles_d, kT, k_pv, g)
        for g in range(1, 4):
            emit_load_group(q_tiles_d, qT, None, g)

        # ---- Phase C: attention over q tiles, software-pipelined ----
        def emit_S(i, after=None):
            """S matmuls (4 separate PSUM chunk tiles) + chunk maxes + negmax."""
            chunks = []
            m4 = small.tile([P, NCH], FP32, tag="m4")
            negmax = small.tile([P, 1], FP32, tag="negmax")
            last_mm = None
            for c in range(NCH):
                psc = ps_s.tile([P, 512], FP32, tag="s")
                for dc in range(NDC):
                    last_mm = nc.tensor.matmul(
                        psc[:],
                        lhsT=qT[:, i, dc, :],
                        rhs=kT[:, c * 4 : (c + 1) * 4, dc, :],
                        start=(dc == 0),
                        stop=(dc == NDC - 1),
                    )
                    if after is not None:
                        tile.add_dep_helper(
                            last_mm.ins, after.ins, False, "S-after-prev-PV"
                        )
                        after = None
                nc.vector.reduce_max(
                    m4[:, c : c + 1], psc[:], axis=mybir.AxisListType.X
                )
                chunks.append(psc)
            nc.vector.reduce_max(
                negmax[:], m4[:], axis=mybir.AxisListType.X, negate=True
            )
            return chunks, negmax, last_mm

        def emit_E(i, chunks, negmax):
            """exp(S - max) per chunk -> p (fp16) + partial row-sums; per-chunk
            DMA xbar transpose into pT right behind each exp; 1/sum via DVE."""
            p = work.tile([P, NCH, 512], pv_dtype, tag="p")
            pT = work.tile([P, NKT, P], pv_dtype, tag="pT")
            rs4 = small.tile([P, NCH], FP32, tag="rs4")
            rowsum = small.tile([P, 1], FP32, tag="rowsum")
            rinv = small.tile([P, 1], FP32, tag="rinv")
            assert NCH % pt_split == 0
            cs = NCH // pt_split  # chunks per transpose
            for c in range(NCH):
                nc.scalar.activation(
                    p[:, c, :], chunks[c][:], AF.Exp, bias=negmax[:],
                    accum_out=rs4[:, c : c + 1],
                )
                if (c + 1) % cs == 0:
                    c0 = c + 1 - cs
                    nc.sync.dma_start_transpose(
                        pT[:, c0 * 4 : (c + 1) * 4, :],
                        p[:, c0 : c + 1, :].rearrange("p c x -> p (c x)"),
                    )
            nc.vector.reduce_sum(rowsum[:], rs4[:], axis=mybir.AxisListType.X)
            nc.vector.reciprocal(rinv[:], rowsum[:])
            return pT, rinv

        def emit_PV(i, pT, rinv, after=None):
            psum_o = ps_pv.tile([P, 512], FP32, tag="pv")
            for t in range(NKT):
                mm = nc.tensor.matmul(
                    psum_o[:],
                    lhsT=pT[:, t, :],
                    rhs=k_pv[:, t],
                    start=(t == 0),
                    stop=(t == NKT - 1),
                )
                if t == 0 and after is not None:
                    # Keep PV(i) behind S(i+1) on the PE queue so PV's work
                    # hides the max->exp->transpose latency of tile i+1.
                    tile.add_dep_helper(
                        mm.ins, after.ins, False, "pv-after-next-S"
                    )
            out_sb = work.tile([P, 512], FP32, tag="out_sb")
            nc.vector.tensor_scalar_mul(out_sb[:], psum_o[:], rinv[:])
            nc.sync.dma_start(out_tiles_d[i], out_sb[:])
            return mm

        def emit_C():
            if depth2:
                state = {}
                for j in (0, 1):
                    s_ps, s_nm, _ = emit_S(j)
                    state[j] = emit_E(j, s_ps, s_nm)
                for i in range(NQT):
                    pT, rinv = state.pop(i)
                    last_pv = emit_PV(i, pT, rinv)
                    if i + 2 < NQT:
                        s_ps, s_nm, _ = emit_S(i + 2, after=last_pv)
                        state[i + 2] = emit_E(i + 2, s_ps, s_nm)
                return
            state = {}
            chunks, negmax, _ = emit_S(0)
            state[0] = emit_E(0, chunks, negmax)
            for i in range(NQT):
                pT, rinv = state.pop(i)
                after = None
                if i + 1 < NQT:
                    s_ps, s_nm, after = emit_S(i + 1)
                    state[i + 1] = emit_E(i + 1, s_ps, s_nm)
                emit_PV(i, pT, rinv, after=after)

        for _ in range(repeat_c):
            emit_C()

        if reps_rv is not None:
            # loop_unroll phase-Cs per iteration: amortizes the For_i
            # all-engine barrier and lets consecutive phase-Cs software-
            # pipeline (as they would in a steady-state workload).
            with tc.For_i(0, reps_rv, 1, staggered_reset=staggered):
                for _ in range(loop_unroll):
                    emit_C()


_NC_CACHE = {}


def _get_nc(**kw):
    key = tuple(sorted((k, str(v)) for k, v in kw.items()))
    if key not in _NC_CACHE:
        _NC_CACHE[key] = build(**kw)
    return _NC_CACHE[key]


def kernel(query: np.ndarray, key: np.ndarray) -> np.ndarray:
    query = np.asarray(query, dtype=np.float32)
    key = np.asarray(key, dtype=np.float32)
    assert query.shape == (B, NQ, D) and key.shape == (B, NK, D)
    nc = _get_nc()
    in_maps = [{"query": query[b], "key": key[b]} for b in range(B)]
    res = run_bass_kernel_spmd(nc, in_maps, list(range(B)))
    return np.stack([res.results[b]["out"] for b in range(B)], axis=0)
